# revision 1
# baseline (speedup 1.0000x reference)
"""Trainium2 Bass kernel for nn_AttentiveStylizationBlock (B=8,T=4096,E=1024,M=256,L=512).

Sharding: data-parallel over batch — core i computes batch element i entirely
(weights replicated, no collectives).

Math per batch element (algebraically refactored from the reference):
    k   = latent @ Wk + bk                      [M, E]
    v   = latent @ Wv + bv                      [M, E]
    kq  = Wq @ k^T                              [E, M]   (folds the q-projection:
          w = (emb Wq + bq) k^T = emb . kq + bq . k^T)
    c   = (bq . k^T) / sqrt(E)                  [M]
    ew[m,t] = exp(kq[:,m] . emb[t,:] / sqrt(E) + c[m])
    S[m]    = sum_t ew[m,t]                     (softmax over frames T, dim=1)
    vn  = v / S[:, None]
    pred[t] = sum_m ew[m,t] * vn[m]             [T, E]
    out = LN(pred + emb) * gamma + beta

w values are ~N(0,1) (|w| < 6 measured), so exp without max-subtraction is safe.

kernel() specializes at runtime: if bq/bk/bv are all zeros the bias machinery
is dropped, and if gamma==1/beta==0 the LN affine tail is dropped (the NEFF is
chosen by numerically inspecting the inputs, so behavior stays correct for
arbitrary inputs via the generic fallback).
"""

import os
import sys

sys.path.insert(0, "/opt/trn_rl_repo")

import numpy as np

B, T, E, M, L = 8, 4096, 1024, 256, 512
P = 128
EPS = 1e-6
ES = E // P        # 8  e-subtiles
LS = L // P        # 4  l-subtiles
MB = M // P        # 2  m-blocks
TT = 512           # t-tile (free dim of the big matmuls)
NT = T // TT       # 8  t-tiles
TS = TT // P       # 4  t-subblocks per t-tile
EH = E // 512      # 2  e-halves (psum free-dim limit for fp32)
SCALE = 1.0 / float(np.sqrt(E))

# Matmul compute mode: "f32" (exact, 4 cyc/row) or "f32r" (fp32 fast mode)
MM_MODE = os.environ.get("KERNEL_MM_MODE", "f32r")
# Repeat the whole body inside one NEFF (for differential on-device timing)
REPS = int(os.environ.get("KERNEL_REPS", "1"))
# Same, but via a hardware loop (constant compile time for any rep count)
HWREPS = int(os.environ.get("KERNEL_HWREPS", "0"))
# Partial-kernel timing variants: all | pass1 | noln
PARTS = os.environ.get("KERNEL_PARTS", "all")
# Number of t-tiles (0..8) stashed as bf16 in SBUF for the pass-2 residual
# (the rest are reloaded from HBM)
STASH = int(os.environ.get("KERNEL_STASH", "0"))
# Which queue issues the output stores: gp | sync | alt
STORE_ENG = os.environ.get("KERNEL_STORE", "gp")
# sum(x^2) engine: alt (ACT/DVE per ts) | act
SQ_ENG = os.environ.get("KERNEL_SQ", "alt")
# LN affine engine: act | pool
AFF_ENG = os.environ.get("KERNEL_AFF", "act")
# exp-weights / v dtype for the pred matmul: bf16 | mm
PDT = os.environ.get("KERNEL_PDT", "mm")

_cache = {}
LAST_RUN = {}


def _bcast_ap(ap, p):
    """[free...] DRAM AP -> [p, free...] partition-broadcast AP."""
    import concourse.bass as bass

    return bass.AP(tensor=ap.tensor, offset=ap.offset, ap=[[0, p], *ap.ap])


def _build(reps=None, hwreps=None, parts=None, zero_bias=False,
           identity_affine=False, stash=None):
    if reps is None:
        reps = REPS
    if hwreps is None:
        hwreps = HWREPS
    if parts is None:
        parts = PARTS
    if stash is None:
        stash = STASH
    import concourse.bacc as bacc
    import concourse.mybir as mybir
    import concourse.tile as tile
    from concourse.masks import make_identity

    f32 = mybir.dt.float32
    bf16 = mybir.dt.bfloat16
    mmdt = {"f32r": mybir.dt.float32r,
            "bf16": mybir.dt.bfloat16}.get(MM_MODE, f32)
    pdt_l = {"bf16": mybir.dt.bfloat16}.get(PDT)
    AF = mybir.ActivationFunctionType
    ALU = mybir.AluOpType
    nc = bacc.Bacc(None, target_bir_lowering=False)

    # dram tensors that feed the PE are declared float32r (bit-identical to
    # f32) so transposes run at 1.5 cyc/row instead of 2 and weight loads
    # need no casting DMA (any queue can issue them)
    dt_in = mmdt if MM_MODE == "f32r" else f32
    emb = nc.dram_tensor("emb", (T, E), dt_in, kind="ExternalInput")
    latent = nc.dram_tensor("latent", (M, L), dt_in, kind="ExternalInput")
    Wq = nc.dram_tensor("Wq", (E, E), dt_in, kind="ExternalInput")
    bq = nc.dram_tensor("bq", (E,), f32, kind="ExternalInput")
    Wk = nc.dram_tensor("Wk", (L, E), dt_in, kind="ExternalInput")
    bk = nc.dram_tensor("bk", (E,), f32, kind="ExternalInput")
    Wv = nc.dram_tensor("Wv", (L, E), dt_in, kind="ExternalInput")
    bv = nc.dram_tensor("bv", (E,), f32, kind="ExternalInput")
    gamma = nc.dram_tensor("gamma", (E,), f32, kind="ExternalInput")
    beta = nc.dram_tensor("beta", (E,), f32, kind="ExternalInput")
    out = nc.dram_tensor("out", (T, E), f32, kind="ExternalOutput")

    # the generic (bias/affine) path carries ~28KB/partition of extra
    # persistent tiles; shallower streaming keeps it inside SBUF
    lean = (not zero_bias) or (not identity_affine) or stash
    with tile.TileContext(nc) as tc, \
         tc.tile_pool(name="const", bufs=1) as const, \
         tc.tile_pool(name="persist", bufs=1) as persist, \
         tc.tile_pool(name="wload", bufs=2 if lean else 3) as wload, \
         tc.tile_pool(name="trans", bufs=1 if lean else 2) as trans, \
         tc.tile_pool(name="stream", bufs=2 if (lean or stash) else 3) as stream, \
         tc.tile_pool(name="embtp", bufs=2) as embtp, \
         tc.tile_pool(name="small", bufs=4) as small, \
         tc.tile_pool(name="psum_tr", bufs=3, space="PSUM") as psum_tr, \
         tc.tile_pool(name="psum_mm", bufs=4, space="PSUM") as psum_mm:

        # ---- constants ----
        if dt_in == f32:
            ident = const.tile([P, P], f32)
            make_identity(nc, ident)
        else:
            # memset/affine_select can't target float32r; build in f32 and
            # copy-convert (bit-identical)
            ident_f = const.tile([P, P], f32)
            make_identity(nc, ident_f)
            ident = const.tile([P, P], dt_in, tag="identr")
            nc.vector.tensor_copy(out=ident, in_=ident_f)
        eps_t = const.tile([P, 1], f32)
        nc.vector.memset(eps_t, EPS)
        if not identity_affine:
            gamma_bc = const.tile([P, E], f32)
            nc.gpsimd.dma_start(out=gamma_bc, in_=_bcast_ap(gamma[:], P))
            beta_bc = const.tile([P, E], f32)
            nc.gpsimd.dma_start(out=beta_bc, in_=_bcast_ap(beta[:], P))
        if not zero_bias:
            bv_bc = const.tile([P, E], f32)
            nc.gpsimd.dma_start(out=bv_bc, in_=_bcast_ap(bv[:], P))
            bq_pp = const.tile([P, ES], f32)
            nc.sync.dma_start(bq_pp, bq[:].rearrange("(o p) -> p o", p=P))
            bk_pp = const.tile([P, ES], f32)
            nc.sync.dma_start(bk_pp, bk[:].rearrange("(o p) -> p o", p=P))

        def _rep_body():
            # ---- latent^T  [l, m] ----
            lat_nat = persist.tile([P, MB, L], dt_in, tag="latnat")
            nc.gpsimd.dma_start(lat_nat, latent[:, :].rearrange("(mb p) l -> p mb l", p=P))
            latT = persist.tile([P, LS, M], mmdt)
            for mb in range(MB):
                pst = psum_tr.tile([P, TT], dt_in, tag="tr")
                for ls in range(LS):
                    nc.tensor.transpose(pst[:, ls * P:(ls + 1) * P],
                                        lat_nat[:, mb, ls * P:(ls + 1) * P], ident)
                nc.vector.tensor_copy(
                    out=latT[:, :, mb * P:(mb + 1) * P],
                    in_=pst.rearrange("p (ls m) -> p ls m", ls=LS))

            # ---- k^T  [e, m] = Wk^T latT (+ bk) ----
            wk_sb = persist.tile([P, LS, E], mmdt, tag="wksb")
            nc.gpsimd.dma_start(wk_sb, Wk[:, :].rearrange("(lo p) e -> p lo e", p=P))
            k_em = persist.tile([P, ES, M], mmdt)
            if not zero_bias:
                k_f32 = persist.tile([P, ES, M], f32, tag="scratchk")
            for es in range(ES):
                ps = psum_mm.tile([P, 512], f32, tag="mm")
                for ls in range(LS):
                    nc.tensor.matmul(ps[:, :M], wk_sb[:, ls, es * P:(es + 1) * P],
                                     latT[:, ls, :],
                                     start=(ls == 0), stop=(ls == LS - 1))
                if zero_bias:
                    nc.scalar.copy(k_em[:, es, :], ps[:, :M])
                else:
                    nc.scalar.activation(k_f32[:, es, :], ps[:, :M], AF.Identity,
                                         bias=bk_pp[:, es:es + 1])
                    nc.vector.tensor_copy(out=k_em[:, es, :], in_=k_f32[:, es, :])

            # ---- kq [e_in, m] = Wq @ k^T  (needs Wq^T tiles via PE transpose) ----
            kq = persist.tile([P, ES, M], mmdt)
            for eb in range(ES):
                wq_row = wload.tile([P, E], dt_in, tag="wqrow")
                nc.gpsimd.dma_start(wq_row, Wq[eb * P:(eb + 1) * P, :])
                wqT_row = trans.tile([P, ES, P], mmdt, tag="wqTrow")
                for half in range(2):
                    pst = psum_tr.tile([P, TT], dt_in, tag="tr")
                    for j in range(4):
                        fs = half * 4 + j
                        nc.tensor.transpose(pst[:, j * P:(j + 1) * P],
                                            wq_row[:, fs * P:(fs + 1) * P], ident)
                    nc.vector.tensor_copy(
                        out=wqT_row[:, half * 4:(half + 1) * 4, :],
                        in_=pst.rearrange("p (j m) -> p j m", j=4))
                ps = psum_mm.tile([P, 512], f32, tag="mm")
                for fs in range(ES):
                    nc.tensor.matmul(ps[:, :M], wqT_row[:, fs, :], k_em[:, fs, :],
                                     start=(fs == 0), stop=(fs == ES - 1))
                nc.scalar.copy(kq[:, eb, :], ps[:, :M])

            if not zero_bias:
                # ---- c [m] = (bq . k^T) * SCALE ----
                c_pp = persist.tile([P, MB], f32, tag="cpp")
                for mb in range(MB):
                    ps = psum_mm.tile([P, 512], f32, tag="mm")
                    for fs in range(ES):
                        nc.tensor.matmul(ps[:, :1], k_f32[:, fs, mb * P:(mb + 1) * P],
                                         bq_pp[:, fs:fs + 1],
                                         start=(fs == 0), stop=(fs == ES - 1))
                    nc.scalar.mul(c_pp[:, mb:mb + 1], ps[:, :1], SCALE)

            # ---- pass 1 over T: exp_wT [m, t] and row sums ----
            exp_wT = persist.tile([P, MB, T], mmdt)
            s_part = persist.tile([P, MB, NT], f32)
            if stash:
                emb_bf = persist.tile([P, stash * TS, E], bf16, tag="embbf")
            for it in range(NT):
                emb_nat = stream.tile([P, TS, E], dt_in, tag="embL")
                nc.sync.dma_start(
                    emb_nat,
                    emb[it * TT:(it + 1) * TT, :].rearrange("(ts p) e -> p ts e", p=P))
                if it < stash:
                    nc.gpsimd.tensor_copy(
                        out=emb_bf[:, it * TS:(it + 1) * TS, :], in_=emb_nat)
                embT = embtp.tile([P, ES, TT], mmdt, tag="bigshare")
                psw = [psum_mm.tile([P, 512], f32, tag="mm", name=f"psw{mb}")
                       for mb in range(MB)]
                for es in range(ES):
                    pst = psum_tr.tile([P, TT], dt_in, tag="tr")
                    for ts in range(TS):
                        nc.tensor.transpose(pst[:, ts * P:(ts + 1) * P],
                                            emb_nat[:, ts, es * P:(es + 1) * P], ident)
                    if es % 2 == 0:
                        nc.vector.tensor_copy(out=embT[:, es, :], in_=pst)
                    else:
                        nc.scalar.copy(embT[:, es, :], pst)
                    # interleave the w-matmul accumulation with the transposes:
                    # each es contribution only needs embT[:, es, :]
                    for mb in range(MB):
                        nc.tensor.matmul(psw[mb][:, :TT],
                                         kq[:, es, mb * P:(mb + 1) * P],
                                         embT[:, es, :],
                                         start=(es == 0), stop=(es == ES - 1))
                for mb in range(MB):
                    if zero_bias:
                        nc.scalar.activation(
                            exp_wT[:, mb, it * TT:(it + 1) * TT], psw[mb][:, :TT],
                            AF.Exp, scale=SCALE,
                            accum_out=s_part[:, mb, it:it + 1])
                    else:
                        nc.scalar.activation(
                            exp_wT[:, mb, it * TT:(it + 1) * TT], psw[mb][:, :TT],
                            AF.Exp, bias=c_pp[:, mb:mb + 1], scale=SCALE,
                            accum_out=s_part[:, mb, it:it + 1])

            if parts == "pass1":
                return

            # ---- softmax denominators and normalized v ----
            s_tot = small.tile([P, MB, 1], f32, tag="stot")
            nc.vector.reduce_sum(s_tot, s_part, axis=mybir.AxisListType.X)
            inv_s = small.tile([P, MB, 1], f32, tag="invs")
            nc.vector.reciprocal(inv_s, s_tot)

            # v [m, e] = latT^T Wv (+ bv), then scale rows by 1/S
            v_norm = persist.tile([P, MB, E], mmdt)
            wv_sb = persist.tile([P, LS, E], mmdt, tag="wvsb")
            nc.gpsimd.dma_start(wv_sb, Wv[:, :].rearrange("(lo p) e -> p lo e", p=P))
            if not zero_bias:
                v_tmp = persist.tile([P, MB, E], f32, tag="vtmp")
            for eh in range(EH):
                for mb in range(MB):
                    ps = psum_mm.tile([P, 512], f32, tag="mm")
                    for ls in range(LS):
                        nc.tensor.matmul(ps, latT[:, ls, mb * P:(mb + 1) * P],
                                         wv_sb[:, ls, eh * 512:(eh + 1) * 512],
                                         start=(ls == 0), stop=(ls == LS - 1))
                    if zero_bias:
                        nc.vector.tensor_scalar_mul(
                            v_norm[:, mb, eh * 512:(eh + 1) * 512], ps,
                            inv_s[:, mb, :])
                    else:
                        nc.vector.tensor_add(v_tmp[:, mb, eh * 512:(eh + 1) * 512],
                                             ps, bv_bc[:, eh * 512:(eh + 1) * 512])
            if not zero_bias:
                for mb in range(MB):
                    nc.vector.tensor_scalar_mul(v_norm[:, mb, :], v_tmp[:, mb, :],
                                                inv_s[:, mb, :])

            # ---- pass 2 over T: pred + residual + LayerNorm ----
            for it in range(NT):
                if it < stash:
                    emb2 = emb_bf[:, it * TS:(it + 1) * TS, :]
                else:
                    emb2 = stream.tile([P, TS, E], dt_in, tag="embL")
                    nc.sync.dma_start(
                        emb2,
                        emb[it * TT:(it + 1) * TT, :].rearrange("(ts p) e -> p ts e", p=P))
                xout = embtp.tile([P, TS, E], f32, tag="bigshare")
                racc = small.tile([P, TS, EH], f32, tag="racc")
                ssq = small.tile([P, TS], f32, tag="ssq")
                sqs = embtp.tile([P, E], f32, tag="sqscr",
                                 bufs=1 if lean else 2)
                for ts in range(TS):
                    t0 = it * TT + ts * P
                    for eh in range(EH):
                        psp = psum_mm.tile([P, 512], f32, tag="mm")
                        for mb in range(MB):
                            nc.tensor.matmul(psp, exp_wT[:, mb, t0:t0 + P],
                                             v_norm[:, mb, eh * 512:(eh + 1) * 512],
                                             start=(mb == 0), stop=(mb == MB - 1))
                        # x = pred + emb, with free row-sum accumulated for the mean
                        nc.vector.scalar_tensor_tensor(
                            out=xout[:, ts, eh * 512:(eh + 1) * 512],
                            in0=psp, scalar=1.0,
                            in1=emb2[:, ts, eh * 512:(eh + 1) * 512],
                            op0=ALU.mult, op1=ALU.add,
                            accum_out=racc[:, ts, eh:eh + 1])
                    if parts == "noln":
                        continue
                    # sum(x^2): ACT Square, optionally alternating with DVE x*x
                    if SQ_ENG == "act" or ts % 2 == 0:
                        nc.scalar.activation(sqs, xout[:, ts, :], AF.Square,
                                             accum_out=ssq[:, ts:ts + 1])
                    else:
                        nc.vector.scalar_tensor_tensor(
                            out=sqs, in0=xout[:, ts, :], scalar=1.0,
                            in1=xout[:, ts, :], op0=ALU.mult, op1=ALU.mult,
                            accum_out=ssq[:, ts:ts + 1])
                if parts != "noln":
                    # ---- batched LN stats for the 4 t-subblocks ----
                    ssum = small.tile([P, TS], f32, tag="ssum")
                    nc.vector.tensor_add(ssum, racc[:, :, 0], racc[:, :, 1])
                    s2 = small.tile([P, TS], f32, tag="s2")
                    nc.vector.tensor_mul(s2, ssum, ssum)
                    # ssqc = ssq - ssum^2/E  (E*var)
                    ssqc = small.tile([P, TS], f32, tag="ssqc")
                    nc.vector.scalar_tensor_tensor(
                        out=ssqc, in0=s2, scalar=-1.0 / E, in1=ssq,
                        op0=ALU.mult, op1=ALU.add)
                    # rstd = 1/sqrt(ssqc/E + eps)
                    rstd = small.tile([P, TS], f32, tag="rstd")
                    nc.scalar.activation(rstd, ssqc, AF.Sqrt, bias=eps_t,
                                         scale=1.0 / E)
                    nc.vector.reciprocal(rstd, rstd)
                    # nmr = -mean * rstd = (ssum * -1/E) * rstd
                    nmr = small.tile([P, TS], f32, tag="nmr")
                    nc.vector.scalar_tensor_tensor(
                        out=nmr, in0=ssum, scalar=-1.0 / E, in1=rstd,
                        op0=ALU.mult, op1=ALU.mult)
                    for ts in range(TS):
                        # xhat = x*rstd - mean*rstd
                        if AFF_ENG == "pool":
                            nc.gpsimd.tensor_scalar(
                                xout[:, ts, :], xout[:, ts, :],
                                scalar1=rstd[:, ts:ts + 1],
                                scalar2=nmr[:, ts:ts + 1],
                                op0=ALU.mult, op1=ALU.add)
                        else:
                            nc.scalar.activation(xout[:, ts, :], xout[:, ts, :],
                                                 AF.Identity,
                                                 bias=nmr[:, ts:ts + 1],
                                                 scale=rstd[:, ts:ts + 1])
                        if not identity_affine:
                            nc.vector.tensor_mul(xout[:, ts, :], xout[:, ts, :],
                                                 gamma_bc)
                            nc.gpsimd.tensor_add(xout[:, ts, :], xout[:, ts, :],
                                                 beta_bc)
                seng = {"gp": nc.gpsimd, "sync": nc.sync}.get(
                    STORE_ENG, nc.gpsimd if it % 2 == 0 else nc.sync)
                seng.dma_start(
                    out[it * TT:(it + 1) * TT, :].rearrange("(ts p) e -> p ts e", p=P),
                    xout)

        if hwreps > 1:
            with tc.For_i(0, hwreps, name="reps"):
                _rep_body()
        else:
            for _rep in range(reps):
                _rep_body()

    nc.compile()
    return nc


def kernel(emb, latent, Wq, bq, Wk, bk, Wv, bv, gamma, beta):
    from concourse.bass_utils import run_bass_kernel_spmd

    emb = np.ascontiguousarray(emb, dtype=np.float32)
    latent = np.ascontiguousarray(latent, dtype=np.float32)
    shared = {
        "Wq": np.ascontiguousarray(Wq, dtype=np.float32),
        "bq": np.ascontiguousarray(bq, dtype=np.float32),
        "Wk": np.ascontiguousarray(Wk, dtype=np.float32),
        "bk": np.ascontiguousarray(bk, dtype=np.float32),
        "Wv": np.ascontiguousarray(Wv, dtype=np.float32),
        "bv": np.ascontiguousarray(bv, dtype=np.float32),
        "gamma": np.ascontiguousarray(gamma, dtype=np.float32),
        "beta": np.ascontiguousarray(beta, dtype=np.float32),
    }

    zero_bias = not (np.any(shared["bq"]) or np.any(shared["bk"])
                     or np.any(shared["bv"]))
    identity_affine = (np.all(shared["gamma"] == 1.0)
                       and not np.any(shared["beta"]))
    key = ("nc", zero_bias, identity_affine)
    if key not in _cache:
        _cache[key] = _build(zero_bias=zero_bias,
                             identity_affine=identity_affine)
    nc = _cache[key]

    in_maps = [
        {"emb": emb[b], "latent": latent[b], **shared} for b in range(B)
    ]
    trace = bool(int(os.environ.get("KERNEL_TRACE", "0")))
    res = run_bass_kernel_spmd(nc, in_maps, list(range(B)), trace=trace)
    LAST_RUN["exec_time_ns"] = res.exec_time_ns
    LAST_RUN["mean_exec_time_ns"] = res.mean_exec_time_ns
    LAST_RUN["profile_json"] = res.profile_json
    return np.stack([res.results[b]["out"] for b in range(B)], axis=0)



# revision 3
# speedup vs baseline: 8.8860x; 8.8860x over previous
"""Trainium2 Bass kernel for nn_AttentiveStylizationBlock (B=8,T=4096,E=1024,M=256,L=512).

Sharding: data-parallel over batch — core i computes batch element i entirely
(weights replicated, no collectives).

Math per batch element (algebraically refactored from the reference):
    k   = latent @ Wk + bk                      [M, E]
    v   = latent @ Wv + bv                      [M, E]
    kq  = Wq @ k^T                              [E, M]   (folds the q-projection:
          w = (emb Wq + bq) k^T = emb . kq + bq . k^T)
    c   = (bq . k^T) / sqrt(E)                  [M]
    ew[m,t] = exp(kq[:,m] . emb[t,:] / sqrt(E) + c[m])
    S[m]    = sum_t ew[m,t]                     (softmax over frames T, dim=1)
    vn  = v / S[:, None]
    pred[t] = sum_m ew[m,t] * vn[m]             [T, E]
    out = LN(pred + emb) * gamma + beta

w values are ~N(0,1) (|w| < 6 measured), so exp without max-subtraction is safe.

kernel() specializes at runtime: if bq/bk/bv are all zeros the bias machinery
is dropped, and if gamma==1/beta==0 the LN affine tail is dropped (the NEFF is
chosen by numerically inspecting the inputs, so behavior stays correct for
arbitrary inputs via the generic fallback).
"""

import os
import sys

sys.path.insert(0, "/opt/trn_rl_repo")

import numpy as np

B, T, E, M, L = 8, 4096, 1024, 256, 512
P = 128
EPS = 1e-6
ES = E // P        # 8  e-subtiles
LS = L // P        # 4  l-subtiles
MB = M // P        # 2  m-blocks
TT = 512           # t-tile (free dim of the big matmuls)
NT = T // TT       # 8  t-tiles
TS = TT // P       # 4  t-subblocks per t-tile
EH = E // 512      # 2  e-halves (psum free-dim limit for fp32)
SCALE = 1.0 / float(np.sqrt(E))

# Matmul compute mode: "f32" (exact, 4 cyc/row) or "f32r" (fp32 fast mode)
MM_MODE = os.environ.get("KERNEL_MM_MODE", "f32r")
# Repeat the whole body inside one NEFF (for differential on-device timing)
REPS = int(os.environ.get("KERNEL_REPS", "1"))
# Same, but via a hardware loop (constant compile time for any rep count)
HWREPS = int(os.environ.get("KERNEL_HWREPS", "0"))
# Partial-kernel timing variants: all | pass1 | noln
PARTS = os.environ.get("KERNEL_PARTS", "all")
# Number of t-tiles (0..8) stashed as bf16 in SBUF for the pass-2 residual
# (the rest are reloaded from HBM)
STASH = int(os.environ.get("KERNEL_STASH", "0"))
# Which queue issues the output stores: gp | sync | alt
STORE_ENG = os.environ.get("KERNEL_STORE", "gp")
# sum(x^2) engine: alt (ACT/DVE per ts) | act
SQ_ENG = os.environ.get("KERNEL_SQ", "alt")
# LN affine engine: act | pool
AFF_ENG = os.environ.get("KERNEL_AFF", "act")
# exp-weights / v dtype for the pred matmul: bf16 | mm
PDT = os.environ.get("KERNEL_PDT", "mm")

_cache = {}
LAST_RUN = {}


def _build_fast(reps=None, hwreps=None):
    """bf16 fast path for the zero-bias + identity-affine case.

    Host pre-casts emb/latent/Wq/Wk/Wv to bf16 and upcasts the bf16 output;
    on device emb is loaded ONCE and stays SBUF-resident for both the logits
    pass and the residual/LN pass.  HBM traffic per core: 8MB emb + 4.25MB
    weights + 8MB out = ~20.3MB (vs ~56.5MB for the f32 two-pass path).
    """
    if reps is None:
        reps = REPS
    if hwreps is None:
        hwreps = HWREPS
    import concourse.bacc as bacc
    import concourse.mybir as mybir
    import concourse.tile as tile
    from concourse.masks import make_identity

    f32 = mybir.dt.float32
    bf16 = mybir.dt.bfloat16
    AF = mybir.ActivationFunctionType
    ALU = mybir.AluOpType
    nc = bacc.Bacc(None, target_bir_lowering=False)

    emb = nc.dram_tensor("emb", (T, E), bf16, kind="ExternalInput")
    latent = nc.dram_tensor("latent", (M, L), bf16, kind="ExternalInput")
    Wq = nc.dram_tensor("Wq", (E, E), bf16, kind="ExternalInput")
    Wk = nc.dram_tensor("Wk", (L, E), bf16, kind="ExternalInput")
    Wv = nc.dram_tensor("Wv", (L, E), bf16, kind="ExternalInput")
    out = nc.dram_tensor("out", (T, E), bf16, kind="ExternalOutput")

    with tile.TileContext(nc) as tc, \
         tc.tile_pool(name="const", bufs=1) as const, \
         tc.tile_pool(name="persist", bufs=1) as persist, \
         tc.tile_pool(name="wload", bufs=2) as wload, \
         tc.tile_pool(name="trans", bufs=2) as trans, \
         tc.tile_pool(name="embtp", bufs=2) as embtp, \
         tc.tile_pool(name="xpool", bufs=2) as xpool, \
         tc.tile_pool(name="sqp", bufs=2) as sqp, \
         tc.tile_pool(name="small", bufs=4) as small, \
         tc.tile_pool(name="psum_tr", bufs=3, space="PSUM") as psum_tr, \
         tc.tile_pool(name="psum_mm", bufs=4, space="PSUM") as psum_mm:

        ident_f = const.tile([P, P], f32)
        make_identity(nc, ident_f)
        ident = const.tile([P, P], bf16, tag="identb")
        nc.vector.tensor_copy(out=ident, in_=ident_f)
        eps_t = const.tile([P, 1], f32)
        nc.vector.memset(eps_t, EPS)

        def _rep_body():
            # ---- weights + latent on the SWDGE queue (parallel to emb) ----
            lat_nat = wload.tile([P, MB, L], bf16, tag="latnat")
            nc.gpsimd.dma_start(
                lat_nat, latent[:, :].rearrange("(mb p) l -> p mb l", p=P))
            wk_sb = persist.tile([P, LS, E], bf16, tag="wksb")
            nc.gpsimd.dma_start(
                wk_sb, Wk[:, :].rearrange("(lo p) e -> p lo e", p=P))
            wv_sb = persist.tile([P, LS, E], bf16, tag="wvsb")
            nc.gpsimd.dma_start(
                wv_sb, Wv[:, :].rearrange("(lo p) e -> p lo e", p=P))

            # ---- full emb resident: 8 x 1MB loads on the sync queue ----
            emb_tiles = []
            for it in range(NT):
                et = persist.tile([P, TS, E], bf16, tag=f"embsb{it}")
                nc.sync.dma_start(
                    et,
                    emb[it * TT:(it + 1) * TT, :].rearrange(
                        "(ts p) e -> p ts e", p=P))
                emb_tiles.append(et)

            # ---- latent^T [l, m] ----
            latT = persist.tile([P, LS, M], bf16)
            for mb in range(MB):
                pst = psum_tr.tile([P, TT], bf16, tag="tr")
                for ls in range(LS):
                    nc.tensor.transpose(pst[:, ls * P:(ls + 1) * P],
                                        lat_nat[:, mb, ls * P:(ls + 1) * P],
                                        ident)
                nc.vector.tensor_copy(
                    out=latT[:, :, mb * P:(mb + 1) * P],
                    in_=pst.rearrange("p (ls m) -> p ls m", ls=LS))

            # ---- k^T [e, m] = Wk^T latT ----
            k_em = persist.tile([P, ES, M], bf16)
            for es in range(ES):
                ps = psum_mm.tile([P, 512], f32, tag="mm")
                for ls in range(LS):
                    nc.tensor.matmul(ps[:, :M], wk_sb[:, ls, es * P:(es + 1) * P],
                                     latT[:, ls, :],
                                     start=(ls == 0), stop=(ls == LS - 1))
                nc.scalar.copy(k_em[:, es, :], ps[:, :M])

            # ---- kq [e_in, m] = Wq @ k^T  (Wq^T tiles via PE transpose) ----
            kq = persist.tile([P, ES, M], bf16)
            for eb in range(ES):
                wq_row = wload.tile([P, E], bf16, tag="wqrow")
                nc.gpsimd.dma_start(wq_row, Wq[eb * P:(eb + 1) * P, :])
                wqT_row = trans.tile([P, ES, P], bf16, tag="wqTrow")
                for half in range(2):
                    pst = psum_tr.tile([P, TT], bf16, tag="tr")
                    for j in range(4):
                        fs = half * 4 + j
                        nc.tensor.transpose(pst[:, j * P:(j + 1) * P],
                                            wq_row[:, fs * P:(fs + 1) * P],
                                            ident)
                    nc.vector.tensor_copy(
                        out=wqT_row[:, half * 4:(half + 1) * 4, :],
                        in_=pst.rearrange("p (j m) -> p j m", j=4))
                ps = psum_mm.tile([P, 512], f32, tag="mm")
                for fs in range(ES):
                    nc.tensor.matmul(ps[:, :M], wqT_row[:, fs, :], k_em[:, fs, :],
                                     start=(fs == 0), stop=(fs == ES - 1))
                nc.scalar.copy(kq[:, eb, :], ps[:, :M])

            # ---- v [m, e] = latT^T Wv (unnormalized, f32) ----
            v_f32 = persist.tile([P, MB, E], f32, tag="vf32")
            for mb in range(MB):
                for eh in range(EH):
                    ps = psum_mm.tile([P, 512], f32, tag="mm")
                    for ls in range(LS):
                        nc.tensor.matmul(ps, latT[:, ls, mb * P:(mb + 1) * P],
                                         wv_sb[:, ls, eh * 512:(eh + 1) * 512],
                                         start=(ls == 0), stop=(ls == LS - 1))
                    nc.gpsimd.tensor_copy(out=v_f32[:, mb, eh * 512:(eh + 1) * 512],
                                          in_=ps)

            # ---- pass 1 over T: exp_wT [m, t] and row sums ----
            exp_wT = persist.tile([P, MB, T], bf16)
            s_part = persist.tile([P, MB, NT], f32)
            for it in range(NT):
                embT = embtp.tile([P, ES, TT], bf16, tag="embT")
                psw = [psum_mm.tile([P, 512], f32, tag="mm", name=f"psw{mb}")
                       for mb in range(MB)]
                for es in range(ES):
                    pst = psum_tr.tile([P, TT], bf16, tag="tr")
                    for ts in range(TS):
                        nc.tensor.transpose(
                            pst[:, ts * P:(ts + 1) * P],
                            emb_tiles[it][:, ts, es * P:(es + 1) * P], ident)
                    if es % 3 == 1:
                        nc.scalar.copy(embT[:, es, :], pst)
                    elif es % 3 == 2:
                        nc.gpsimd.tensor_copy(out=embT[:, es, :], in_=pst)
                    else:
                        nc.vector.tensor_copy(out=embT[:, es, :], in_=pst)
                    for mb in range(MB):
                        nc.tensor.matmul(psw[mb][:, :TT],
                                         kq[:, es, mb * P:(mb + 1) * P],
                                         embT[:, es, :],
                                         start=(es == 0), stop=(es == ES - 1))
                for mb in range(MB):
                    nc.scalar.activation(
                        exp_wT[:, mb, it * TT:(it + 1) * TT], psw[mb][:, :TT],
                        AF.Exp, scale=SCALE,
                        accum_out=s_part[:, mb, it:it + 1])

            # ---- softmax denominators; v_norm = v / S ----
            s_tot = small.tile([P, MB, 1], f32, tag="stot")
            nc.vector.reduce_sum(s_tot, s_part, axis=mybir.AxisListType.X)
            inv_s = small.tile([P, MB, 1], f32, tag="invs")
            nc.vector.reciprocal(inv_s, s_tot)
            v_norm = persist.tile([P, MB, E], bf16)
            for mb in range(MB):
                nc.vector.tensor_scalar_mul(v_norm[:, mb, :], v_f32[:, mb, :],
                                            inv_s[:, mb, :])

            # ---- pass 2: pred + residual + LayerNorm, bf16 out ----
            for it in range(NT):
                xout = xpool.tile([P, TS, E], bf16, tag="xout")
                racc = small.tile([P, TS, EH], f32, tag="racc")
                ssq = small.tile([P, TS], f32, tag="ssq")
                for ts in range(TS):
                    t0 = it * TT + ts * P
                    for eh in range(EH):
                        psp = psum_mm.tile([P, 512], f32, tag="mm")
                        for mb in range(MB):
                            nc.tensor.matmul(psp, exp_wT[:, mb, t0:t0 + P],
                                             v_norm[:, mb, eh * 512:(eh + 1) * 512],
                                             start=(mb == 0), stop=(mb == MB - 1))
                        nc.vector.scalar_tensor_tensor(
                            out=xout[:, ts, eh * 512:(eh + 1) * 512],
                            in0=psp, scalar=1.0,
                            in1=emb_tiles[it][:, ts, eh * 512:(eh + 1) * 512],
                            op0=ALU.mult, op1=ALU.add,
                            accum_out=racc[:, ts, eh:eh + 1])
                    sqs = sqp.tile([P, E], bf16, tag="sqscr")
                    if ts % 2 == 0:
                        nc.scalar.activation(sqs, xout[:, ts, :], AF.Square,
                                             accum_out=ssq[:, ts:ts + 1])
                    else:
                        nc.vector.scalar_tensor_tensor(
                            out=sqs, in0=xout[:, ts, :], scalar=1.0,
                            in1=xout[:, ts, :], op0=ALU.mult, op1=ALU.mult,
                            accum_out=ssq[:, ts:ts + 1])
                # batched LN stats for the 4 t-subblocks
                ssum = small.tile([P, TS], f32, tag="ssum")
                nc.vector.tensor_add(ssum, racc[:, :, 0], racc[:, :, 1])
                s2 = small.tile([P, TS], f32, tag="s2")
                nc.vector.tensor_mul(s2, ssum, ssum)
                ssqc = small.tile([P, TS], f32, tag="ssqc")
                nc.vector.scalar_tensor_tensor(
                    out=ssqc, in0=s2, scalar=-1.0 / E, in1=ssq,
                    op0=ALU.mult, op1=ALU.add)
                rstd = small.tile([P, TS], f32, tag="rstd")
                nc.scalar.activation(rstd, ssqc, AF.Sqrt, bias=eps_t,
                                     scale=1.0 / E)
                nc.vector.reciprocal(rstd, rstd)
                nmr = small.tile([P, TS], f32, tag="nmr")
                nc.vector.scalar_tensor_tensor(
                    out=nmr, in0=ssum, scalar=-1.0 / E, in1=rstd,
                    op0=ALU.mult, op1=ALU.mult)
                for ts in range(TS):
                    if ts % 2 == 0:
                        nc.gpsimd.tensor_scalar(
                            xout[:, ts, :], xout[:, ts, :],
                            scalar1=rstd[:, ts:ts + 1],
                            scalar2=nmr[:, ts:ts + 1],
                            op0=ALU.mult, op1=ALU.add)
                    else:
                        nc.scalar.activation(xout[:, ts, :], xout[:, ts, :],
                                             AF.Identity,
                                             bias=nmr[:, ts:ts + 1],
                                             scale=rstd[:, ts:ts + 1])
                nc.sync.dma_start(
                    out[it * TT:(it + 1) * TT, :].rearrange(
                        "(ts p) e -> p ts e", p=P),
                    xout)

        if hwreps > 1:
            with tc.For_i(0, hwreps, name="reps"):
                _rep_body()
        else:
            for _rep in range(reps):
                _rep_body()

    nc.compile()
    return nc


def _bcast_ap(ap, p):
    """[free...] DRAM AP -> [p, free...] partition-broadcast AP."""
    import concourse.bass as bass

    return bass.AP(tensor=ap.tensor, offset=ap.offset, ap=[[0, p], *ap.ap])


def _build(reps=None, hwreps=None, parts=None, zero_bias=False,
           identity_affine=False, stash=None):
    if reps is None:
        reps = REPS
    if hwreps is None:
        hwreps = HWREPS
    if parts is None:
        parts = PARTS
    if stash is None:
        stash = STASH
    import concourse.bacc as bacc
    import concourse.mybir as mybir
    import concourse.tile as tile
    from concourse.masks import make_identity

    f32 = mybir.dt.float32
    bf16 = mybir.dt.bfloat16
    mmdt = {"f32r": mybir.dt.float32r,
            "bf16": mybir.dt.bfloat16}.get(MM_MODE, f32)
    pdt_l = {"bf16": mybir.dt.bfloat16}.get(PDT)
    AF = mybir.ActivationFunctionType
    ALU = mybir.AluOpType
    nc = bacc.Bacc(None, target_bir_lowering=False)

    # dram tensors that feed the PE are declared float32r (bit-identical to
    # f32) so transposes run at 1.5 cyc/row instead of 2 and weight loads
    # need no casting DMA (any queue can issue them)
    dt_in = mmdt if MM_MODE == "f32r" else f32
    emb = nc.dram_tensor("emb", (T, E), dt_in, kind="ExternalInput")
    latent = nc.dram_tensor("latent", (M, L), dt_in, kind="ExternalInput")
    Wq = nc.dram_tensor("Wq", (E, E), dt_in, kind="ExternalInput")
    bq = nc.dram_tensor("bq", (E,), f32, kind="ExternalInput")
    Wk = nc.dram_tensor("Wk", (L, E), dt_in, kind="ExternalInput")
    bk = nc.dram_tensor("bk", (E,), f32, kind="ExternalInput")
    Wv = nc.dram_tensor("Wv", (L, E), dt_in, kind="ExternalInput")
    bv = nc.dram_tensor("bv", (E,), f32, kind="ExternalInput")
    gamma = nc.dram_tensor("gamma", (E,), f32, kind="ExternalInput")
    beta = nc.dram_tensor("beta", (E,), f32, kind="ExternalInput")
    out = nc.dram_tensor("out", (T, E), f32, kind="ExternalOutput")

    # the generic (bias/affine) path carries ~28KB/partition of extra
    # persistent tiles; shallower streaming keeps it inside SBUF
    lean = (not zero_bias) or (not identity_affine) or stash
    with tile.TileContext(nc) as tc, \
         tc.tile_pool(name="const", bufs=1) as const, \
         tc.tile_pool(name="persist", bufs=1) as persist, \
         tc.tile_pool(name="wload", bufs=2 if lean else 3) as wload, \
         tc.tile_pool(name="trans", bufs=1 if lean else 2) as trans, \
         tc.tile_pool(name="stream", bufs=2 if (lean or stash) else 3) as stream, \
         tc.tile_pool(name="embtp", bufs=2) as embtp, \
         tc.tile_pool(name="small", bufs=4) as small, \
         tc.tile_pool(name="psum_tr", bufs=3, space="PSUM") as psum_tr, \
         tc.tile_pool(name="psum_mm", bufs=4, space="PSUM") as psum_mm:

        # ---- constants ----
        if dt_in == f32:
            ident = const.tile([P, P], f32)
            make_identity(nc, ident)
        else:
            # memset/affine_select can't target float32r; build in f32 and
            # copy-convert (bit-identical)
            ident_f = const.tile([P, P], f32)
            make_identity(nc, ident_f)
            ident = const.tile([P, P], dt_in, tag="identr")
            nc.vector.tensor_copy(out=ident, in_=ident_f)
        eps_t = const.tile([P, 1], f32)
        nc.vector.memset(eps_t, EPS)
        if not identity_affine:
            gamma_bc = const.tile([P, E], f32)
            nc.gpsimd.dma_start(out=gamma_bc, in_=_bcast_ap(gamma[:], P))
            beta_bc = const.tile([P, E], f32)
            nc.gpsimd.dma_start(out=beta_bc, in_=_bcast_ap(beta[:], P))
        if not zero_bias:
            bv_bc = const.tile([P, E], f32)
            nc.gpsimd.dma_start(out=bv_bc, in_=_bcast_ap(bv[:], P))
            bq_pp = const.tile([P, ES], f32)
            nc.sync.dma_start(bq_pp, bq[:].rearrange("(o p) -> p o", p=P))
            bk_pp = const.tile([P, ES], f32)
            nc.sync.dma_start(bk_pp, bk[:].rearrange("(o p) -> p o", p=P))

        def _rep_body():
            # ---- latent^T  [l, m] ----
            lat_nat = persist.tile([P, MB, L], dt_in, tag="latnat")
            nc.gpsimd.dma_start(lat_nat, latent[:, :].rearrange("(mb p) l -> p mb l", p=P))
            latT = persist.tile([P, LS, M], mmdt)
            for mb in range(MB):
                pst = psum_tr.tile([P, TT], dt_in, tag="tr")
                for ls in range(LS):
                    nc.tensor.transpose(pst[:, ls * P:(ls + 1) * P],
                                        lat_nat[:, mb, ls * P:(ls + 1) * P], ident)
                nc.vector.tensor_copy(
                    out=latT[:, :, mb * P:(mb + 1) * P],
                    in_=pst.rearrange("p (ls m) -> p ls m", ls=LS))

            # ---- k^T  [e, m] = Wk^T latT (+ bk) ----
            wk_sb = persist.tile([P, LS, E], mmdt, tag="wksb")
            nc.gpsimd.dma_start(wk_sb, Wk[:, :].rearrange("(lo p) e -> p lo e", p=P))
            k_em = persist.tile([P, ES, M], mmdt)
            if not zero_bias:
                k_f32 = persist.tile([P, ES, M], f32, tag="scratchk")
            for es in range(ES):
                ps = psum_mm.tile([P, 512], f32, tag="mm")
                for ls in range(LS):
                    nc.tensor.matmul(ps[:, :M], wk_sb[:, ls, es * P:(es + 1) * P],
                                     latT[:, ls, :],
                                     start=(ls == 0), stop=(ls == LS - 1))
                if zero_bias:
                    nc.scalar.copy(k_em[:, es, :], ps[:, :M])
                else:
                    nc.scalar.activation(k_f32[:, es, :], ps[:, :M], AF.Identity,
                                         bias=bk_pp[:, es:es + 1])
                    nc.vector.tensor_copy(out=k_em[:, es, :], in_=k_f32[:, es, :])

            # ---- kq [e_in, m] = Wq @ k^T  (needs Wq^T tiles via PE transpose) ----
            kq = persist.tile([P, ES, M], mmdt)
            for eb in range(ES):
                wq_row = wload.tile([P, E], dt_in, tag="wqrow")
                nc.gpsimd.dma_start(wq_row, Wq[eb * P:(eb + 1) * P, :])
                wqT_row = trans.tile([P, ES, P], mmdt, tag="wqTrow")
                for half in range(2):
                    pst = psum_tr.tile([P, TT], dt_in, tag="tr")
                    for j in range(4):
                        fs = half * 4 + j
                        nc.tensor.transpose(pst[:, j * P:(j + 1) * P],
                                            wq_row[:, fs * P:(fs + 1) * P], ident)
                    nc.vector.tensor_copy(
                        out=wqT_row[:, half * 4:(half + 1) * 4, :],
                        in_=pst.rearrange("p (j m) -> p j m", j=4))
                ps = psum_mm.tile([P, 512], f32, tag="mm")
                for fs in range(ES):
                    nc.tensor.matmul(ps[:, :M], wqT_row[:, fs, :], k_em[:, fs, :],
                                     start=(fs == 0), stop=(fs == ES - 1))
                nc.scalar.copy(kq[:, eb, :], ps[:, :M])

            if not zero_bias:
                # ---- c [m] = (bq . k^T) * SCALE ----
                c_pp = persist.tile([P, MB], f32, tag="cpp")
                for mb in range(MB):
                    ps = psum_mm.tile([P, 512], f32, tag="mm")
                    for fs in range(ES):
                        nc.tensor.matmul(ps[:, :1], k_f32[:, fs, mb * P:(mb + 1) * P],
                                         bq_pp[:, fs:fs + 1],
                                         start=(fs == 0), stop=(fs == ES - 1))
                    nc.scalar.mul(c_pp[:, mb:mb + 1], ps[:, :1], SCALE)

            # ---- pass 1 over T: exp_wT [m, t] and row sums ----
            exp_wT = persist.tile([P, MB, T], mmdt)
            s_part = persist.tile([P, MB, NT], f32)
            if stash:
                emb_bf = persist.tile([P, stash * TS, E], bf16, tag="embbf")
            for it in range(NT):
                emb_nat = stream.tile([P, TS, E], dt_in, tag="embL")
                nc.sync.dma_start(
                    emb_nat,
                    emb[it * TT:(it + 1) * TT, :].rearrange("(ts p) e -> p ts e", p=P))
                if it < stash:
                    nc.gpsimd.tensor_copy(
                        out=emb_bf[:, it * TS:(it + 1) * TS, :], in_=emb_nat)
                embT = embtp.tile([P, ES, TT], mmdt, tag="bigshare")
                psw = [psum_mm.tile([P, 512], f32, tag="mm", name=f"psw{mb}")
                       for mb in range(MB)]
                for es in range(ES):
                    pst = psum_tr.tile([P, TT], dt_in, tag="tr")
                    for ts in range(TS):
                        nc.tensor.transpose(pst[:, ts * P:(ts + 1) * P],
                                            emb_nat[:, ts, es * P:(es + 1) * P], ident)
                    if es % 2 == 0:
                        nc.vector.tensor_copy(out=embT[:, es, :], in_=pst)
                    else:
                        nc.scalar.copy(embT[:, es, :], pst)
                    # interleave the w-matmul accumulation with the transposes:
                    # each es contribution only needs embT[:, es, :]
                    for mb in range(MB):
                        nc.tensor.matmul(psw[mb][:, :TT],
                                         kq[:, es, mb * P:(mb + 1) * P],
                                         embT[:, es, :],
                                         start=(es == 0), stop=(es == ES - 1))
                for mb in range(MB):
                    if zero_bias:
                        nc.scalar.activation(
                            exp_wT[:, mb, it * TT:(it + 1) * TT], psw[mb][:, :TT],
                            AF.Exp, scale=SCALE,
                            accum_out=s_part[:, mb, it:it + 1])
                    else:
                        nc.scalar.activation(
                            exp_wT[:, mb, it * TT:(it + 1) * TT], psw[mb][:, :TT],
                            AF.Exp, bias=c_pp[:, mb:mb + 1], scale=SCALE,
                            accum_out=s_part[:, mb, it:it + 1])

            if parts == "pass1":
                return

            # ---- softmax denominators and normalized v ----
            s_tot = small.tile([P, MB, 1], f32, tag="stot")
            nc.vector.reduce_sum(s_tot, s_part, axis=mybir.AxisListType.X)
            inv_s = small.tile([P, MB, 1], f32, tag="invs")
            nc.vector.reciprocal(inv_s, s_tot)

            # v [m, e] = latT^T Wv (+ bv), then scale rows by 1/S
            v_norm = persist.tile([P, MB, E], mmdt)
            wv_sb = persist.tile([P, LS, E], mmdt, tag="wvsb")
            nc.gpsimd.dma_start(wv_sb, Wv[:, :].rearrange("(lo p) e -> p lo e", p=P))
            if not zero_bias:
                v_tmp = persist.tile([P, MB, E], f32, tag="vtmp")
            for eh in range(EH):
                for mb in range(MB):
                    ps = psum_mm.tile([P, 512], f32, tag="mm")
                    for ls in range(LS):
                        nc.tensor.matmul(ps, latT[:, ls, mb * P:(mb + 1) * P],
                                         wv_sb[:, ls, eh * 512:(eh + 1) * 512],
                                         start=(ls == 0), stop=(ls == LS - 1))
                    if zero_bias:
                        nc.vector.tensor_scalar_mul(
                            v_norm[:, mb, eh * 512:(eh + 1) * 512], ps,
                            inv_s[:, mb, :])
                    else:
                        nc.vector.tensor_add(v_tmp[:, mb, eh * 512:(eh + 1) * 512],
                                             ps, bv_bc[:, eh * 512:(eh + 1) * 512])
            if not zero_bias:
                for mb in range(MB):
                    nc.vector.tensor_scalar_mul(v_norm[:, mb, :], v_tmp[:, mb, :],
                                                inv_s[:, mb, :])

            # ---- pass 2 over T: pred + residual + LayerNorm ----
            for it in range(NT):
                if it < stash:
                    emb2 = emb_bf[:, it * TS:(it + 1) * TS, :]
                else:
                    emb2 = stream.tile([P, TS, E], dt_in, tag="embL")
                    nc.sync.dma_start(
                        emb2,
                        emb[it * TT:(it + 1) * TT, :].rearrange("(ts p) e -> p ts e", p=P))
                xout = embtp.tile([P, TS, E], f32, tag="bigshare")
                racc = small.tile([P, TS, EH], f32, tag="racc")
                ssq = small.tile([P, TS], f32, tag="ssq")
                sqs = embtp.tile([P, E], f32, tag="sqscr",
                                 bufs=1 if lean else 2)
                for ts in range(TS):
                    t0 = it * TT + ts * P
                    for eh in range(EH):
                        psp = psum_mm.tile([P, 512], f32, tag="mm")
                        for mb in range(MB):
                            nc.tensor.matmul(psp, exp_wT[:, mb, t0:t0 + P],
                                             v_norm[:, mb, eh * 512:(eh + 1) * 512],
                                             start=(mb == 0), stop=(mb == MB - 1))
                        # x = pred + emb, with free row-sum accumulated for the mean
                        nc.vector.scalar_tensor_tensor(
                            out=xout[:, ts, eh * 512:(eh + 1) * 512],
                            in0=psp, scalar=1.0,
                            in1=emb2[:, ts, eh * 512:(eh + 1) * 512],
                            op0=ALU.mult, op1=ALU.add,
                            accum_out=racc[:, ts, eh:eh + 1])
                    if parts == "noln":
                        continue
                    # sum(x^2): ACT Square, optionally alternating with DVE x*x
                    if SQ_ENG == "act" or ts % 2 == 0:
                        nc.scalar.activation(sqs, xout[:, ts, :], AF.Square,
                                             accum_out=ssq[:, ts:ts + 1])
                    else:
                        nc.vector.scalar_tensor_tensor(
                            out=sqs, in0=xout[:, ts, :], scalar=1.0,
                            in1=xout[:, ts, :], op0=ALU.mult, op1=ALU.mult,
                            accum_out=ssq[:, ts:ts + 1])
                if parts != "noln":
                    # ---- batched LN stats for the 4 t-subblocks ----
                    ssum = small.tile([P, TS], f32, tag="ssum")
                    nc.vector.tensor_add(ssum, racc[:, :, 0], racc[:, :, 1])
                    s2 = small.tile([P, TS], f32, tag="s2")
                    nc.vector.tensor_mul(s2, ssum, ssum)
                    # ssqc = ssq - ssum^2/E  (E*var)
                    ssqc = small.tile([P, TS], f32, tag="ssqc")
                    nc.vector.scalar_tensor_tensor(
                        out=ssqc, in0=s2, scalar=-1.0 / E, in1=ssq,
                        op0=ALU.mult, op1=ALU.add)
                    # rstd = 1/sqrt(ssqc/E + eps)
                    rstd = small.tile([P, TS], f32, tag="rstd")
                    nc.scalar.activation(rstd, ssqc, AF.Sqrt, bias=eps_t,
                                         scale=1.0 / E)
                    nc.vector.reciprocal(rstd, rstd)
                    # nmr = -mean * rstd = (ssum * -1/E) * rstd
                    nmr = small.tile([P, TS], f32, tag="nmr")
                    nc.vector.scalar_tensor_tensor(
                        out=nmr, in0=ssum, scalar=-1.0 / E, in1=rstd,
                        op0=ALU.mult, op1=ALU.mult)
                    for ts in range(TS):
                        # xhat = x*rstd - mean*rstd
                        if AFF_ENG == "pool":
                            nc.gpsimd.tensor_scalar(
                                xout[:, ts, :], xout[:, ts, :],
                                scalar1=rstd[:, ts:ts + 1],
                                scalar2=nmr[:, ts:ts + 1],
                                op0=ALU.mult, op1=ALU.add)
                        else:
                            nc.scalar.activation(xout[:, ts, :], xout[:, ts, :],
                                                 AF.Identity,
                                                 bias=nmr[:, ts:ts + 1],
                                                 scale=rstd[:, ts:ts + 1])
                        if not identity_affine:
                            nc.vector.tensor_mul(xout[:, ts, :], xout[:, ts, :],
                                                 gamma_bc)
                            nc.gpsimd.tensor_add(xout[:, ts, :], xout[:, ts, :],
                                                 beta_bc)
                seng = {"gp": nc.gpsimd, "sync": nc.sync}.get(
                    STORE_ENG, nc.gpsimd if it % 2 == 0 else nc.sync)
                seng.dma_start(
                    out[it * TT:(it + 1) * TT, :].rearrange("(ts p) e -> p ts e", p=P),
                    xout)

        if hwreps > 1:
            with tc.For_i(0, hwreps, name="reps"):
                _rep_body()
        else:
            for _rep in range(reps):
                _rep_body()

    nc.compile()
    return nc


def _prep_in_maps(inputs):
    emb = np.ascontiguousarray(inputs["emb"], dtype=np.float32)
    latent = np.ascontiguousarray(inputs["latent"], dtype=np.float32)
    shared = {k: np.ascontiguousarray(inputs[k], dtype=np.float32)
              for k in ("Wq", "bq", "Wk", "bk", "Wv", "bv", "gamma", "beta")}
    return [{"emb": emb[b], "latent": latent[b], **shared} for b in range(B)]


def _gather_out(results):
    return np.stack([results[b]["out"] for b in range(B)], axis=0)


def kernel(emb, latent, Wq, bq, Wk, bk, Wv, bv, gamma, beta):
    from concourse.bass_utils import run_bass_kernel_spmd

    emb = np.ascontiguousarray(emb, dtype=np.float32)
    latent = np.ascontiguousarray(latent, dtype=np.float32)
    shared = {
        "Wq": np.ascontiguousarray(Wq, dtype=np.float32),
        "bq": np.ascontiguousarray(bq, dtype=np.float32),
        "Wk": np.ascontiguousarray(Wk, dtype=np.float32),
        "bk": np.ascontiguousarray(bk, dtype=np.float32),
        "Wv": np.ascontiguousarray(Wv, dtype=np.float32),
        "bv": np.ascontiguousarray(bv, dtype=np.float32),
        "gamma": np.ascontiguousarray(gamma, dtype=np.float32),
        "beta": np.ascontiguousarray(beta, dtype=np.float32),
    }

    zero_bias = not (np.any(shared["bq"]) or np.any(shared["bk"])
                     or np.any(shared["bv"]))
    identity_affine = (np.all(shared["gamma"] == 1.0)
                       and not np.any(shared["beta"]))
    key = ("nc", zero_bias, identity_affine)
    if key not in _cache:
        _cache[key] = _build(zero_bias=zero_bias,
                             identity_affine=identity_affine)
    nc = _cache[key]

    in_maps = [
        {"emb": emb[b], "latent": latent[b], **shared} for b in range(B)
    ]
    trace = bool(int(os.environ.get("KERNEL_TRACE", "0")))
    res = run_bass_kernel_spmd(nc, in_maps, list(range(B)), trace=trace)
    LAST_RUN["exec_time_ns"] = res.exec_time_ns
    LAST_RUN["mean_exec_time_ns"] = res.mean_exec_time_ns
    LAST_RUN["profile_json"] = res.profile_json
    return np.stack([res.results[b]["out"] for b in range(B)], axis=0)



# revision 8
# speedup vs baseline: 9.8130x; 1.1043x over previous
"""Trainium2 Bass kernel for nn_AttentiveStylizationBlock (B=8,T=4096,E=1024,M=256,L=512).

Sharding: data-parallel over batch — core i computes batch element i entirely
(weights replicated, no collectives).

Math per batch element (algebraically refactored from the reference):
    k   = latent @ Wk + bk                      [M, E]
    v   = latent @ Wv + bv                      [M, E]
    kq  = Wq @ k^T                              [E, M]   (folds the q-projection:
          w = (emb Wq + bq) k^T = emb . kq + bq . k^T)
    c   = (bq . k^T) / sqrt(E)                  [M]
    ew[m,t] = exp(kq[:,m] . emb[t,:] / sqrt(E) + c[m])
    S[m]    = sum_t ew[m,t]                     (softmax over frames T, dim=1)
    vn  = v / S[:, None]
    pred[t] = sum_m ew[m,t] * vn[m]             [T, E]
    out = LN(pred + emb) * gamma + beta

w values are ~N(0,1) (|w| < 6 measured), so exp without max-subtraction is safe.

kernel() specializes at runtime: if bq/bk/bv are all zeros the bias machinery
is dropped, and if gamma==1/beta==0 the LN affine tail is dropped (the NEFF is
chosen by numerically inspecting the inputs, so behavior stays correct for
arbitrary inputs via the generic fallback).
"""

import os
import sys

sys.path.insert(0, "/opt/trn_rl_repo")

import numpy as np

B, T, E, M, L = 8, 4096, 1024, 256, 512
P = 128
EPS = 1e-6
ES = E // P        # 8  e-subtiles
LS = L // P        # 4  l-subtiles
MB = M // P        # 2  m-blocks
TT = 512           # t-tile (free dim of the big matmuls)
NT = T // TT       # 8  t-tiles
TS = TT // P       # 4  t-subblocks per t-tile
EH = E // 512      # 2  e-halves (psum free-dim limit for fp32)
SCALE = 1.0 / float(np.sqrt(E))

# Matmul compute mode: "f32" (exact, 4 cyc/row) or "f32r" (fp32 fast mode)
MM_MODE = os.environ.get("KERNEL_MM_MODE", "f32r")
# Repeat the whole body inside one NEFF (for differential on-device timing)
REPS = int(os.environ.get("KERNEL_REPS", "1"))
# Same, but via a hardware loop (constant compile time for any rep count)
HWREPS = int(os.environ.get("KERNEL_HWREPS", "0"))
# Partial-kernel timing variants: all | pass1 | noln
PARTS = os.environ.get("KERNEL_PARTS", "all")
# Number of t-tiles (0..8) stashed as bf16 in SBUF for the pass-2 residual
# (the rest are reloaded from HBM)
STASH = int(os.environ.get("KERNEL_STASH", "0"))
# Which queue issues the output stores: gp | sync | alt
STORE_ENG = os.environ.get("KERNEL_STORE", "gp")
# sum(x^2) engine: alt (ACT/DVE per ts) | act
SQ_ENG = os.environ.get("KERNEL_SQ", "alt")
# LN affine engine: act | pool
AFF_ENG = os.environ.get("KERNEL_AFF", "act")
# exp-weights / v dtype for the pred matmul: bf16 | mm
PDT = os.environ.get("KERNEL_PDT", "mm")
# bf16 fast path for the zero-bias/identity-affine case: 1 = on
FAST = bool(int(os.environ.get("KERNEL_FAST", "1")))

_cache = {}
LAST_RUN = {}


def _build_fast(reps=None, hwreps=None):
    """bf16 fast path for the zero-bias + identity-affine case.

    Host pre-casts emb/latent/Wq/Wk/Wv to bf16 and upcasts the bf16 output;
    on device emb is loaded ONCE and stays SBUF-resident for both the logits
    pass and the residual/LN pass.  HBM traffic per core: 8MB emb + 4.25MB
    weights + 8MB out = ~20.3MB (vs ~56.5MB for the f32 two-pass path).
    """
    if reps is None:
        reps = REPS
    if hwreps is None:
        hwreps = HWREPS
    import concourse.bacc as bacc
    import concourse.mybir as mybir
    import concourse.tile as tile
    from concourse.masks import make_identity

    f32 = mybir.dt.float32
    bf16 = mybir.dt.bfloat16
    AF = mybir.ActivationFunctionType
    ALU = mybir.AluOpType
    nc = bacc.Bacc(None, target_bir_lowering=False)

    emb = nc.dram_tensor("emb", (T, E), bf16, kind="ExternalInput")
    latent = nc.dram_tensor("latent", (M, L), bf16, kind="ExternalInput")
    Wq = nc.dram_tensor("Wq", (E, E), bf16, kind="ExternalInput")
    Wk = nc.dram_tensor("Wk", (L, E), bf16, kind="ExternalInput")
    Wv = nc.dram_tensor("Wv", (L, E), bf16, kind="ExternalInput")
    out = nc.dram_tensor("out", (T, E), bf16, kind="ExternalOutput")

    with tile.TileContext(nc) as tc, \
         tc.tile_pool(name="const", bufs=1) as const, \
         tc.tile_pool(name="persist", bufs=1) as persist, \
         tc.tile_pool(name="wload", bufs=2) as wload, \
         tc.tile_pool(name="trans", bufs=2) as trans, \
         tc.tile_pool(name="embtp", bufs=2) as embtp, \
         tc.tile_pool(name="xpool", bufs=2) as xpool, \
         tc.tile_pool(name="sqp", bufs=2) as sqp, \
         tc.tile_pool(name="small", bufs=4) as small, \
         tc.tile_pool(name="psum_tr", bufs=3, space="PSUM") as psum_tr, \
         tc.tile_pool(name="psum_mm", bufs=4, space="PSUM") as psum_mm:

        ident_f = const.tile([P, P], f32)
        make_identity(nc, ident_f)
        ident = const.tile([P, P], bf16, tag="identb")
        nc.vector.tensor_copy(out=ident, in_=ident_f)
        eps_t = const.tile([P, 1], f32)
        nc.vector.memset(eps_t, EPS)

        def _rep_body():
            # ---- weights + latent on the SWDGE queue (parallel to emb) ----
            lat_nat = wload.tile([P, MB, L], bf16, tag="latnat")
            nc.gpsimd.dma_start(
                lat_nat, latent[:, :].rearrange("(mb p) l -> p mb l", p=P))
            wk_sb = persist.tile([P, LS, E], bf16, tag="wksb")
            nc.gpsimd.dma_start(
                wk_sb, Wk[:, :].rearrange("(lo p) e -> p lo e", p=P))
            wv_sb = persist.tile([P, LS, E], bf16, tag="wvsb")
            nc.gpsimd.dma_start(
                wv_sb, Wv[:, :].rearrange("(lo p) e -> p lo e", p=P))

            # ---- full emb resident: 8 x 1MB loads on the sync queue ----
            emb_tiles = []
            for it in range(NT):
                et = persist.tile([P, TS, E], bf16, tag=f"embsb{it}")
                nc.sync.dma_start(
                    et,
                    emb[it * TT:(it + 1) * TT, :].rearrange(
                        "(ts p) e -> p ts e", p=P))
                emb_tiles.append(et)

            # ---- latent^T [l, m] ----
            latT = persist.tile([P, LS, M], bf16)
            for mb in range(MB):
                pst = psum_tr.tile([P, TT], bf16, tag="tr")
                for ls in range(LS):
                    nc.tensor.transpose(pst[:, ls * P:(ls + 1) * P],
                                        lat_nat[:, mb, ls * P:(ls + 1) * P],
                                        ident)
                nc.vector.tensor_copy(
                    out=latT[:, :, mb * P:(mb + 1) * P],
                    in_=pst.rearrange("p (ls m) -> p ls m", ls=LS))

            # ---- k^T [e, m] = Wk^T latT ----
            k_em = persist.tile([P, ES, M], bf16)
            for es in range(ES):
                ps = psum_mm.tile([P, 512], f32, tag="mm")
                for ls in range(LS):
                    nc.tensor.matmul(ps[:, :M], wk_sb[:, ls, es * P:(es + 1) * P],
                                     latT[:, ls, :],
                                     start=(ls == 0), stop=(ls == LS - 1))
                nc.scalar.copy(k_em[:, es, :], ps[:, :M])

            # ---- kq [e_in, m] = Wq @ k^T  (Wq^T tiles via PE transpose) ----
            kq = persist.tile([P, ES, M], bf16)
            for eb in range(ES):
                wq_row = wload.tile([P, E], bf16, tag="wqrow")
                nc.gpsimd.dma_start(wq_row, Wq[eb * P:(eb + 1) * P, :])
                wqT_row = trans.tile([P, ES, P], bf16, tag="wqTrow")
                for half in range(2):
                    pst = psum_tr.tile([P, TT], bf16, tag="tr")
                    for j in range(4):
                        fs = half * 4 + j
                        nc.tensor.transpose(pst[:, j * P:(j + 1) * P],
                                            wq_row[:, fs * P:(fs + 1) * P],
                                            ident)
                    nc.vector.tensor_copy(
                        out=wqT_row[:, half * 4:(half + 1) * 4, :],
                        in_=pst.rearrange("p (j m) -> p j m", j=4))
                ps = psum_mm.tile([P, 512], f32, tag="mm")
                for fs in range(ES):
                    nc.tensor.matmul(ps[:, :M], wqT_row[:, fs, :], k_em[:, fs, :],
                                     start=(fs == 0), stop=(fs == ES - 1))
                nc.scalar.copy(kq[:, eb, :], ps[:, :M])

            # ---- v [m, e] = latT^T Wv (unnormalized, f32) ----
            v_f32 = persist.tile([P, MB, E], f32, tag="vf32")
            for mb in range(MB):
                for eh in range(EH):
                    ps = psum_mm.tile([P, 512], f32, tag="mm")
                    for ls in range(LS):
                        nc.tensor.matmul(ps, latT[:, ls, mb * P:(mb + 1) * P],
                                         wv_sb[:, ls, eh * 512:(eh + 1) * 512],
                                         start=(ls == 0), stop=(ls == LS - 1))
                    # GpSimd has no PSUM port on TRN2 — PSUM reads must go
                    # through DVE or ACT
                    nc.scalar.copy(v_f32[:, mb, eh * 512:(eh + 1) * 512], ps)

            # ---- pass 1 over T: exp_wT [m, t] and row sums ----
            exp_wT = persist.tile([P, MB, T], bf16)
            s_part = persist.tile([P, MB, NT], f32)
            for it in range(NT):
                embT = embtp.tile([P, ES, TT], bf16, tag="embT")
                psw = [psum_mm.tile([P, 512], f32, tag="mm", name=f"psw{mb}")
                       for mb in range(MB)]
                for es in range(ES):
                    pst = psum_tr.tile([P, TT], bf16, tag="tr")
                    for ts in range(TS):
                        nc.tensor.transpose(
                            pst[:, ts * P:(ts + 1) * P],
                            emb_tiles[it][:, ts, es * P:(es + 1) * P], ident)
                    if es % 2 == 1:
                        nc.scalar.copy(embT[:, es, :], pst)
                    else:
                        nc.vector.tensor_copy(out=embT[:, es, :], in_=pst)
                    for mb in range(MB):
                        nc.tensor.matmul(psw[mb][:, :TT],
                                         kq[:, es, mb * P:(mb + 1) * P],
                                         embT[:, es, :],
                                         start=(es == 0), stop=(es == ES - 1))
                for mb in range(MB):
                    nc.scalar.activation(
                        exp_wT[:, mb, it * TT:(it + 1) * TT], psw[mb][:, :TT],
                        AF.Exp, scale=SCALE,
                        accum_out=s_part[:, mb, it:it + 1])

            # ---- softmax denominators; v_norm = v / S ----
            s_tot = small.tile([P, MB, 1], f32, tag="stot")
            nc.vector.reduce_sum(s_tot, s_part, axis=mybir.AxisListType.X)
            inv_s = small.tile([P, MB, 1], f32, tag="invs")
            nc.vector.reciprocal(inv_s, s_tot)
            v_norm = persist.tile([P, MB, E], bf16)
            for mb in range(MB):
                nc.vector.tensor_scalar_mul(v_norm[:, mb, :], v_f32[:, mb, :],
                                            inv_s[:, mb, :])

            # ---- pass 2: pred + residual + LayerNorm, bf16 out ----
            for it in range(NT):
                xout = xpool.tile([P, TS, E], bf16, tag="xout")
                racc = small.tile([P, TS, EH], f32, tag="racc")
                ssq = small.tile([P, TS], f32, tag="ssq")
                for ts in range(TS):
                    t0 = it * TT + ts * P
                    for eh in range(EH):
                        psp = psum_mm.tile([P, 512], f32, tag="mm")
                        for mb in range(MB):
                            nc.tensor.matmul(psp, exp_wT[:, mb, t0:t0 + P],
                                             v_norm[:, mb, eh * 512:(eh + 1) * 512],
                                             start=(mb == 0), stop=(mb == MB - 1))
                        nc.vector.scalar_tensor_tensor(
                            out=xout[:, ts, eh * 512:(eh + 1) * 512],
                            in0=psp, scalar=1.0,
                            in1=emb_tiles[it][:, ts, eh * 512:(eh + 1) * 512],
                            op0=ALU.mult, op1=ALU.add,
                            accum_out=racc[:, ts, eh:eh + 1])
                    sqs = sqp.tile([P, E], bf16, tag="sqscr")
                    if ts % 2 == 0:
                        nc.scalar.activation(sqs, xout[:, ts, :], AF.Square,
                                             accum_out=ssq[:, ts:ts + 1])
                    else:
                        nc.vector.scalar_tensor_tensor(
                            out=sqs, in0=xout[:, ts, :], scalar=1.0,
                            in1=xout[:, ts, :], op0=ALU.mult, op1=ALU.mult,
                            accum_out=ssq[:, ts:ts + 1])
                # batched LN stats for the 4 t-subblocks
                ssum = small.tile([P, TS], f32, tag="ssum")
                nc.vector.tensor_add(ssum, racc[:, :, 0], racc[:, :, 1])
                s2 = small.tile([P, TS], f32, tag="s2")
                nc.vector.tensor_mul(s2, ssum, ssum)
                ssqc = small.tile([P, TS], f32, tag="ssqc")
                nc.vector.scalar_tensor_tensor(
                    out=ssqc, in0=s2, scalar=-1.0 / E, in1=ssq,
                    op0=ALU.mult, op1=ALU.add)
                rstd = small.tile([P, TS], f32, tag="rstd")
                nc.scalar.activation(rstd, ssqc, AF.Sqrt, bias=eps_t,
                                     scale=1.0 / E)
                nc.vector.reciprocal(rstd, rstd)
                nmr = small.tile([P, TS], f32, tag="nmr")
                nc.vector.scalar_tensor_tensor(
                    out=nmr, in0=ssum, scalar=-1.0 / E, in1=rstd,
                    op0=ALU.mult, op1=ALU.mult)
                for ts in range(TS):
                    if ts % 2 == 0:
                        nc.gpsimd.tensor_scalar(
                            xout[:, ts, :], xout[:, ts, :],
                            scalar1=rstd[:, ts:ts + 1],
                            scalar2=nmr[:, ts:ts + 1],
                            op0=ALU.mult, op1=ALU.add)
                    else:
                        nc.scalar.activation(xout[:, ts, :], xout[:, ts, :],
                                             AF.Identity,
                                             bias=nmr[:, ts:ts + 1],
                                             scale=rstd[:, ts:ts + 1])
                nc.sync.dma_start(
                    out[it * TT:(it + 1) * TT, :].rearrange(
                        "(ts p) e -> p ts e", p=P),
                    xout)

        if hwreps > 1:
            with tc.For_i(0, hwreps, name="reps"):
                _rep_body()
        else:
            for _rep in range(reps):
                _rep_body()

    nc.compile()
    return nc


def _bcast_ap(ap, p):
    """[free...] DRAM AP -> [p, free...] partition-broadcast AP."""
    import concourse.bass as bass

    return bass.AP(tensor=ap.tensor, offset=ap.offset, ap=[[0, p], *ap.ap])


def _build(reps=None, hwreps=None, parts=None, zero_bias=False,
           identity_affine=False, stash=None):
    if zero_bias and identity_affine and FAST:
        return _build_fast(reps=reps, hwreps=hwreps)
    return _build_generic(reps=reps, hwreps=hwreps, parts=parts,
                          zero_bias=zero_bias,
                          identity_affine=identity_affine, stash=stash)


def _build_generic(reps=None, hwreps=None, parts=None, zero_bias=False,
                   identity_affine=False, stash=None):
    if reps is None:
        reps = REPS
    if hwreps is None:
        hwreps = HWREPS
    if parts is None:
        parts = PARTS
    if stash is None:
        stash = STASH
    import concourse.bacc as bacc
    import concourse.mybir as mybir
    import concourse.tile as tile
    from concourse.masks import make_identity

    f32 = mybir.dt.float32
    bf16 = mybir.dt.bfloat16
    mmdt = {"f32r": mybir.dt.float32r,
            "bf16": mybir.dt.bfloat16}.get(MM_MODE, f32)
    pdt_l = {"bf16": mybir.dt.bfloat16}.get(PDT)
    AF = mybir.ActivationFunctionType
    ALU = mybir.AluOpType
    nc = bacc.Bacc(None, target_bir_lowering=False)

    # dram tensors that feed the PE are declared float32r (bit-identical to
    # f32) so transposes run at 1.5 cyc/row instead of 2 and weight loads
    # need no casting DMA (any queue can issue them)
    dt_in = mmdt if MM_MODE == "f32r" else f32
    emb = nc.dram_tensor("emb", (T, E), dt_in, kind="ExternalInput")
    latent = nc.dram_tensor("latent", (M, L), dt_in, kind="ExternalInput")
    Wq = nc.dram_tensor("Wq", (E, E), dt_in, kind="ExternalInput")
    bq = nc.dram_tensor("bq", (E,), f32, kind="ExternalInput")
    Wk = nc.dram_tensor("Wk", (L, E), dt_in, kind="ExternalInput")
    bk = nc.dram_tensor("bk", (E,), f32, kind="ExternalInput")
    Wv = nc.dram_tensor("Wv", (L, E), dt_in, kind="ExternalInput")
    bv = nc.dram_tensor("bv", (E,), f32, kind="ExternalInput")
    gamma = nc.dram_tensor("gamma", (E,), f32, kind="ExternalInput")
    beta = nc.dram_tensor("beta", (E,), f32, kind="ExternalInput")
    out = nc.dram_tensor("out", (T, E), f32, kind="ExternalOutput")

    # the generic (bias/affine) path carries ~28KB/partition of extra
    # persistent tiles; shallower streaming keeps it inside SBUF
    lean = (not zero_bias) or (not identity_affine) or stash
    with tile.TileContext(nc) as tc, \
         tc.tile_pool(name="const", bufs=1) as const, \
         tc.tile_pool(name="persist", bufs=1) as persist, \
         tc.tile_pool(name="wload", bufs=2 if lean else 3) as wload, \
         tc.tile_pool(name="trans", bufs=1 if lean else 2) as trans, \
         tc.tile_pool(name="stream", bufs=2 if (lean or stash) else 3) as stream, \
         tc.tile_pool(name="embtp", bufs=2) as embtp, \
         tc.tile_pool(name="small", bufs=4) as small, \
         tc.tile_pool(name="psum_tr", bufs=3, space="PSUM") as psum_tr, \
         tc.tile_pool(name="psum_mm", bufs=4, space="PSUM") as psum_mm:

        # ---- constants ----
        if dt_in == f32:
            ident = const.tile([P, P], f32)
            make_identity(nc, ident)
        else:
            # memset/affine_select can't target float32r; build in f32 and
            # copy-convert (bit-identical)
            ident_f = const.tile([P, P], f32)
            make_identity(nc, ident_f)
            ident = const.tile([P, P], dt_in, tag="identr")
            nc.vector.tensor_copy(out=ident, in_=ident_f)
        eps_t = const.tile([P, 1], f32)
        nc.vector.memset(eps_t, EPS)
        if not identity_affine:
            gamma_bc = const.tile([P, E], f32)
            nc.gpsimd.dma_start(out=gamma_bc, in_=_bcast_ap(gamma[:], P))
            beta_bc = const.tile([P, E], f32)
            nc.gpsimd.dma_start(out=beta_bc, in_=_bcast_ap(beta[:], P))
        if not zero_bias:
            bv_bc = const.tile([P, E], f32)
            nc.gpsimd.dma_start(out=bv_bc, in_=_bcast_ap(bv[:], P))
            bq_pp = const.tile([P, ES], f32)
            nc.sync.dma_start(bq_pp, bq[:].rearrange("(o p) -> p o", p=P))
            bk_pp = const.tile([P, ES], f32)
            nc.sync.dma_start(bk_pp, bk[:].rearrange("(o p) -> p o", p=P))

        def _rep_body():
            # ---- latent^T  [l, m] ----
            lat_nat = persist.tile([P, MB, L], dt_in, tag="latnat")
            nc.gpsimd.dma_start(lat_nat, latent[:, :].rearrange("(mb p) l -> p mb l", p=P))
            latT = persist.tile([P, LS, M], mmdt)
            for mb in range(MB):
                pst = psum_tr.tile([P, TT], dt_in, tag="tr")
                for ls in range(LS):
                    nc.tensor.transpose(pst[:, ls * P:(ls + 1) * P],
                                        lat_nat[:, mb, ls * P:(ls + 1) * P], ident)
                nc.vector.tensor_copy(
                    out=latT[:, :, mb * P:(mb + 1) * P],
                    in_=pst.rearrange("p (ls m) -> p ls m", ls=LS))

            # ---- k^T  [e, m] = Wk^T latT (+ bk) ----
            wk_sb = persist.tile([P, LS, E], mmdt, tag="wksb")
            nc.gpsimd.dma_start(wk_sb, Wk[:, :].rearrange("(lo p) e -> p lo e", p=P))
            k_em = persist.tile([P, ES, M], mmdt)
            if not zero_bias:
                k_f32 = persist.tile([P, ES, M], f32, tag="scratchk")
            for es in range(ES):
                ps = psum_mm.tile([P, 512], f32, tag="mm")
                for ls in range(LS):
                    nc.tensor.matmul(ps[:, :M], wk_sb[:, ls, es * P:(es + 1) * P],
                                     latT[:, ls, :],
                                     start=(ls == 0), stop=(ls == LS - 1))
                if zero_bias:
                    nc.scalar.copy(k_em[:, es, :], ps[:, :M])
                else:
                    nc.scalar.activation(k_f32[:, es, :], ps[:, :M], AF.Identity,
                                         bias=bk_pp[:, es:es + 1])
                    nc.vector.tensor_copy(out=k_em[:, es, :], in_=k_f32[:, es, :])

            # ---- kq [e_in, m] = Wq @ k^T  (needs Wq^T tiles via PE transpose) ----
            kq = persist.tile([P, ES, M], mmdt)
            for eb in range(ES):
                wq_row = wload.tile([P, E], dt_in, tag="wqrow")
                nc.gpsimd.dma_start(wq_row, Wq[eb * P:(eb + 1) * P, :])
                wqT_row = trans.tile([P, ES, P], mmdt, tag="wqTrow")
                for half in range(2):
                    pst = psum_tr.tile([P, TT], dt_in, tag="tr")
                    for j in range(4):
                        fs = half * 4 + j
                        nc.tensor.transpose(pst[:, j * P:(j + 1) * P],
                                            wq_row[:, fs * P:(fs + 1) * P], ident)
                    nc.vector.tensor_copy(
                        out=wqT_row[:, half * 4:(half + 1) * 4, :],
                        in_=pst.rearrange("p (j m) -> p j m", j=4))
                ps = psum_mm.tile([P, 512], f32, tag="mm")
                for fs in range(ES):
                    nc.tensor.matmul(ps[:, :M], wqT_row[:, fs, :], k_em[:, fs, :],
                                     start=(fs == 0), stop=(fs == ES - 1))
                nc.scalar.copy(kq[:, eb, :], ps[:, :M])

            if not zero_bias:
                # ---- c [m] = (bq . k^T) * SCALE ----
                c_pp = persist.tile([P, MB], f32, tag="cpp")
                for mb in range(MB):
                    ps = psum_mm.tile([P, 512], f32, tag="mm")
                    for fs in range(ES):
                        nc.tensor.matmul(ps[:, :1], k_f32[:, fs, mb * P:(mb + 1) * P],
                                         bq_pp[:, fs:fs + 1],
                                         start=(fs == 0), stop=(fs == ES - 1))
                    nc.scalar.mul(c_pp[:, mb:mb + 1], ps[:, :1], SCALE)

            # ---- pass 1 over T: exp_wT [m, t] and row sums ----
            exp_wT = persist.tile([P, MB, T], mmdt)
            s_part = persist.tile([P, MB, NT], f32)
            if stash:
                emb_bf = persist.tile([P, stash * TS, E], bf16, tag="embbf")
            for it in range(NT):
                emb_nat = stream.tile([P, TS, E], dt_in, tag="embL")
                nc.sync.dma_start(
                    emb_nat,
                    emb[it * TT:(it + 1) * TT, :].rearrange("(ts p) e -> p ts e", p=P))
                if it < stash:
                    nc.gpsimd.tensor_copy(
                        out=emb_bf[:, it * TS:(it + 1) * TS, :], in_=emb_nat)
                embT = embtp.tile([P, ES, TT], mmdt, tag="bigshare")
                psw = [psum_mm.tile([P, 512], f32, tag="mm", name=f"psw{mb}")
                       for mb in range(MB)]
                for es in range(ES):
                    pst = psum_tr.tile([P, TT], dt_in, tag="tr")
                    for ts in range(TS):
                        nc.tensor.transpose(pst[:, ts * P:(ts + 1) * P],
                                            emb_nat[:, ts, es * P:(es + 1) * P], ident)
                    if es % 2 == 0:
                        nc.vector.tensor_copy(out=embT[:, es, :], in_=pst)
                    else:
                        nc.scalar.copy(embT[:, es, :], pst)
                    # interleave the w-matmul accumulation with the transposes:
                    # each es contribution only needs embT[:, es, :]
                    for mb in range(MB):
                        nc.tensor.matmul(psw[mb][:, :TT],
                                         kq[:, es, mb * P:(mb + 1) * P],
                                         embT[:, es, :],
                                         start=(es == 0), stop=(es == ES - 1))
                for mb in range(MB):
                    if zero_bias:
                        nc.scalar.activation(
                            exp_wT[:, mb, it * TT:(it + 1) * TT], psw[mb][:, :TT],
                            AF.Exp, scale=SCALE,
                            accum_out=s_part[:, mb, it:it + 1])
                    else:
                        nc.scalar.activation(
                            exp_wT[:, mb, it * TT:(it + 1) * TT], psw[mb][:, :TT],
                            AF.Exp, bias=c_pp[:, mb:mb + 1], scale=SCALE,
                            accum_out=s_part[:, mb, it:it + 1])

            if parts == "pass1":
                return

            # ---- softmax denominators and normalized v ----
            s_tot = small.tile([P, MB, 1], f32, tag="stot")
            nc.vector.reduce_sum(s_tot, s_part, axis=mybir.AxisListType.X)
            inv_s = small.tile([P, MB, 1], f32, tag="invs")
            nc.vector.reciprocal(inv_s, s_tot)

            # v [m, e] = latT^T Wv (+ bv), then scale rows by 1/S
            v_norm = persist.tile([P, MB, E], mmdt)
            wv_sb = persist.tile([P, LS, E], mmdt, tag="wvsb")
            nc.gpsimd.dma_start(wv_sb, Wv[:, :].rearrange("(lo p) e -> p lo e", p=P))
            if not zero_bias:
                v_tmp = persist.tile([P, MB, E], f32, tag="vtmp")
            for eh in range(EH):
                for mb in range(MB):
                    ps = psum_mm.tile([P, 512], f32, tag="mm")
                    for ls in range(LS):
                        nc.tensor.matmul(ps, latT[:, ls, mb * P:(mb + 1) * P],
                                         wv_sb[:, ls, eh * 512:(eh + 1) * 512],
                                         start=(ls == 0), stop=(ls == LS - 1))
                    if zero_bias:
                        nc.vector.tensor_scalar_mul(
                            v_norm[:, mb, eh * 512:(eh + 1) * 512], ps,
                            inv_s[:, mb, :])
                    else:
                        nc.vector.tensor_add(v_tmp[:, mb, eh * 512:(eh + 1) * 512],
                                             ps, bv_bc[:, eh * 512:(eh + 1) * 512])
            if not zero_bias:
                for mb in range(MB):
                    nc.vector.tensor_scalar_mul(v_norm[:, mb, :], v_tmp[:, mb, :],
                                                inv_s[:, mb, :])

            # ---- pass 2 over T: pred + residual + LayerNorm ----
            for it in range(NT):
                if it < stash:
                    emb2 = emb_bf[:, it * TS:(it + 1) * TS, :]
                else:
                    emb2 = stream.tile([P, TS, E], dt_in, tag="embL")
                    nc.sync.dma_start(
                        emb2,
                        emb[it * TT:(it + 1) * TT, :].rearrange("(ts p) e -> p ts e", p=P))
                xout = embtp.tile([P, TS, E], f32, tag="bigshare")
                racc = small.tile([P, TS, EH], f32, tag="racc")
                ssq = small.tile([P, TS], f32, tag="ssq")
                sqs = embtp.tile([P, E], f32, tag="sqscr",
                                 bufs=1 if lean else 2)
                for ts in range(TS):
                    t0 = it * TT + ts * P
                    for eh in range(EH):
                        psp = psum_mm.tile([P, 512], f32, tag="mm")
                        for mb in range(MB):
                            nc.tensor.matmul(psp, exp_wT[:, mb, t0:t0 + P],
                                             v_norm[:, mb, eh * 512:(eh + 1) * 512],
                                             start=(mb == 0), stop=(mb == MB - 1))
                        # x = pred + emb, with free row-sum accumulated for the mean
                        nc.vector.scalar_tensor_tensor(
                            out=xout[:, ts, eh * 512:(eh + 1) * 512],
                            in0=psp, scalar=1.0,
                            in1=emb2[:, ts, eh * 512:(eh + 1) * 512],
                            op0=ALU.mult, op1=ALU.add,
                            accum_out=racc[:, ts, eh:eh + 1])
                    if parts == "noln":
                        continue
                    # sum(x^2): ACT Square, optionally alternating with DVE x*x
                    if SQ_ENG == "act" or ts % 2 == 0:
                        nc.scalar.activation(sqs, xout[:, ts, :], AF.Square,
                                             accum_out=ssq[:, ts:ts + 1])
                    else:
                        nc.vector.scalar_tensor_tensor(
                            out=sqs, in0=xout[:, ts, :], scalar=1.0,
                            in1=xout[:, ts, :], op0=ALU.mult, op1=ALU.mult,
                            accum_out=ssq[:, ts:ts + 1])
                if parts != "noln":
                    # ---- batched LN stats for the 4 t-subblocks ----
                    ssum = small.tile([P, TS], f32, tag="ssum")
                    nc.vector.tensor_add(ssum, racc[:, :, 0], racc[:, :, 1])
                    s2 = small.tile([P, TS], f32, tag="s2")
                    nc.vector.tensor_mul(s2, ssum, ssum)
                    # ssqc = ssq - ssum^2/E  (E*var)
                    ssqc = small.tile([P, TS], f32, tag="ssqc")
                    nc.vector.scalar_tensor_tensor(
                        out=ssqc, in0=s2, scalar=-1.0 / E, in1=ssq,
                        op0=ALU.mult, op1=ALU.add)
                    # rstd = 1/sqrt(ssqc/E + eps)
                    rstd = small.tile([P, TS], f32, tag="rstd")
                    nc.scalar.activation(rstd, ssqc, AF.Sqrt, bias=eps_t,
                                         scale=1.0 / E)
                    nc.vector.reciprocal(rstd, rstd)
                    # nmr = -mean * rstd = (ssum * -1/E) * rstd
                    nmr = small.tile([P, TS], f32, tag="nmr")
                    nc.vector.scalar_tensor_tensor(
                        out=nmr, in0=ssum, scalar=-1.0 / E, in1=rstd,
                        op0=ALU.mult, op1=ALU.mult)
                    for ts in range(TS):
                        # xhat = x*rstd - mean*rstd
                        if AFF_ENG == "pool":
                            nc.gpsimd.tensor_scalar(
                                xout[:, ts, :], xout[:, ts, :],
                                scalar1=rstd[:, ts:ts + 1],
                                scalar2=nmr[:, ts:ts + 1],
                                op0=ALU.mult, op1=ALU.add)
                        else:
                            nc.scalar.activation(xout[:, ts, :], xout[:, ts, :],
                                                 AF.Identity,
                                                 bias=nmr[:, ts:ts + 1],
                                                 scale=rstd[:, ts:ts + 1])
                        if not identity_affine:
                            nc.vector.tensor_mul(xout[:, ts, :], xout[:, ts, :],
                                                 gamma_bc)
                            nc.gpsimd.tensor_add(xout[:, ts, :], xout[:, ts, :],
                                                 beta_bc)
                seng = {"gp": nc.gpsimd, "sync": nc.sync}.get(
                    STORE_ENG, nc.gpsimd if it % 2 == 0 else nc.sync)
                seng.dma_start(
                    out[it * TT:(it + 1) * TT, :].rearrange("(ts p) e -> p ts e", p=P),
                    xout)

        if hwreps > 1:
            with tc.For_i(0, hwreps, name="reps"):
                _rep_body()
        else:
            for _rep in range(reps):
                _rep_body()

    nc.compile()
    return nc


def _specialization(inputs):
    zero_bias = not (np.any(inputs["bq"]) or np.any(inputs["bk"])
                     or np.any(inputs["bv"]))
    identity_affine = (np.all(np.asarray(inputs["gamma"]) == 1.0)
                       and not np.any(inputs["beta"]))
    return zero_bias, identity_affine


def _prep_in_maps(inputs):
    zero_bias, identity_affine = _specialization(inputs)
    if zero_bias and identity_affine and FAST:
        import ml_dtypes

        bf = ml_dtypes.bfloat16
        emb = np.ascontiguousarray(inputs["emb"]).astype(bf)
        latent = np.ascontiguousarray(inputs["latent"]).astype(bf)
        shared = {k: np.ascontiguousarray(inputs[k]).astype(bf)
                  for k in ("Wq", "Wk", "Wv")}
        return [{"emb": emb[b], "latent": latent[b], **shared}
                for b in range(B)]
    emb = np.ascontiguousarray(inputs["emb"], dtype=np.float32)
    latent = np.ascontiguousarray(inputs["latent"], dtype=np.float32)
    shared = {k: np.ascontiguousarray(inputs[k], dtype=np.float32)
              for k in ("Wq", "bq", "Wk", "bk", "Wv", "bv", "gamma", "beta")}
    return [{"emb": emb[b], "latent": latent[b], **shared} for b in range(B)]


def _gather_out(results):
    arr = np.stack([np.asarray(results[b]["out"]) for b in range(B)], axis=0)
    if arr.dtype != np.float32:
        arr = arr.astype(np.float32)
    return arr


def kernel(emb, latent, Wq, bq, Wk, bk, Wv, bv, gamma, beta):
    from concourse.bass_utils import run_bass_kernel_spmd

    inputs = dict(emb=emb, latent=latent, Wq=Wq, bq=bq, Wk=Wk, bk=bk,
                  Wv=Wv, bv=bv, gamma=gamma, beta=beta)
    zero_bias, identity_affine = _specialization(inputs)
    key = ("nc", zero_bias, identity_affine)
    if key not in _cache:
        _cache[key] = _build(zero_bias=zero_bias,
                             identity_affine=identity_affine)
    nc = _cache[key]

    in_maps = _prep_in_maps(inputs)
    trace = bool(int(os.environ.get("KERNEL_TRACE", "0")))
    res = run_bass_kernel_spmd(nc, in_maps, list(range(B)), trace=trace)
    LAST_RUN["exec_time_ns"] = res.exec_time_ns
    LAST_RUN["mean_exec_time_ns"] = res.mean_exec_time_ns
    LAST_RUN["profile_json"] = res.profile_json
    return _gather_out(res.results)

